# revision 19
# baseline (speedup 1.0000x reference)
"""FEELVOS fused kernel for TRN2, 8-core SPMD.

Sharding: the reference only returns logits for classes C-2, C-1, so only 4 of
the 8 fused (batch, class) items matter. 8 cores = 4 (b, c) pairs x 2 frame
halves (top/bottom 24 rows). Bottom-half cores receive row-flipped inputs and
row-flipped conv kernels so every core runs the identical program computing
"top 25 rows" of its (possibly flipped) frame; the host un-flips on gather.

Per core:
  - partial U-Net on x3[b] (top-aligned row windows; convs as im2col or
    9-shifted matmuls on zero-padded [C, 50x50] SBUF planes, fp32r)
  - 100-d embeddings of x1[b,c], x2[b,c] (full frame, ref side) and of the
    class-c channel of the U-Net output (25-row window, query side)
  - distance matrix via one K=101 matmul per tile: rows 0..99 carry
    e3 . (-2 e_ref), row 100 carries 1 * |a_p|^2; DVE reduce_min over the
    free axis; |b_q|^2 added after the min (per-chunk column matmul);
    then tanh(d/2) == 1 - 2*sigmoid(-d)
  - 3x3 head conv (im2col, K=63) over [x3s(4ch), gm, lm, x2] -> 24 rows.
"""
import numpy as np

import concourse.bass as bass
import concourse.bacc as bacc
import concourse.tile as tile
from concourse import mybir
from concourse.bass_utils import run_bass_kernel_spmd
from concourse.masks import make_identity

F32 = mybir.dt.float32
F32R = mybir.dt.float32r
AF = mybir.ActivationFunctionType
ALU = mybir.AluOpType
AX = mybir.AxisListType

H = W = 48
NPIX = H * W                 # 2304 ref pixels
QROWS = 25
Q = QROWS * W                # 1200 query pixels
QCH, NQC = 120, 10           # query chunking for the distance matmul
PCH = [(0, 512), (512, 512), (1024, 512), (1536, 512), (2048, 256)]

_PROG = None


def _r3(ap, h, w):
    return ap.rearrange("c (h w) -> c h w", h=h, w=w)


def _emit(nc, tc, ctx):
    # ------------------------------------------------------------- dram io
    def din(name, shape):
        return nc.dram_tensor(name, shape, F32, kind="ExternalInput").ap()

    d = {}
    d["x1c"] = din("x1c", [H, W])
    d["x2c"] = din("x2c", [H, W])
    d["x3b"] = din("x3b", [3, H, W])
    for nm, sh in [("enc1_wT", [27, 16]), ("enc2_wT", [16, 288]),
                   ("bott_wT", [32, 576]), ("dec2a_wT", [64, 288]),
                   ("dec2b_wT", [32, 288]), ("dec1a_wT", [32, 144]),
                   ("dec1b_wT", [16, 144]), ("out_wT", [16, 4]),
                   ("outc_wT", [16, 1]), ("emb_wT10", [10, 100]),
                   ("dsh_wT", [63, 1]),
                   ("enc1_b", [16]), ("enc2_b", [32]), ("bott_b", [64]),
                   ("dec2_b", [32]), ("dec1_b", [16]), ("out_b", [4]),
                   ("dsh_b", [1]), ("out_bc", [1])]:
        d[nm] = din(nm, sh)
    out_d = nc.dram_tensor("out", [24, W], F32, kind="ExternalOutput").ap()

    # round-robin DMA dispatch over the two HWDGE engines
    _dmaq = [nc.sync, nc.scalar]
    _qi = [0]

    def dma(out, in_):
        eng = _dmaq[_qi[0] % len(_dmaq)]
        _qi[0] += 1
        eng.dma_start(out, in_)

    # ------------------------------------------------------------- sbuf
    sb = ctx.enter_context(tc.tile_pool(name="sb", bufs=1))

    def st(name, p, f, dt=F32):
        return sb.tile([p, f], dt, tag=name, name=name)

    # padded feature planes ([C, (rows+2)*(cols+2)])
    xp3 = st("xp3", 3, 2500, F32R)
    x1p = st("x1p", 1, 2500, F32R)
    x2p = st("x2p", 1, 2500, F32R)
    x3cp = st("x3cp", 1, 2500, F32R)
    e1p = st("e1p", 16, 2500, F32R)
    p1p = st("p1p", 16, 676, F32R)
    e2p = st("e2p", 32, 676, F32R)
    p2p = st("p2p", 32, 196, F32R)
    btp = st("btp", 64, 196, F32R)
    ubp = st("ubp", 64, 676, F32R)
    d2p = st("d2p", 32, 676, F32R)
    udp = st("udp", 32, 2500, F32R)
    d1p = st("d1p", 16, 2500, F32R)
    xt = st("xt", 7, 2500, F32R)

    im2c = st("im2c", 10, NPIX, F32R)       # emb im2col (e1/e2)
    im2c3 = st("im2c3", 10, Q, F32R)        # emb im2col (e3)
    im27 = st("im27", 27, 39 * W, F32R)     # enc1 im2col (rows ci*9+s)
    imdsh = st("imdsh", 63, 24 * W, F32R)   # head im2col
    e1x = st("e1x", 101, NPIX, F32R)
    e2x = st("e2x", 101, NPIX, F32R)
    e3x = st("e3x", 101, Q, F32R)
    esq = st("esq", 100, NPIX, F32R)
    ident = st("ident", 128, 128)

    # transposed weights (fp32r, matmul lhsT)
    wt = {k: st("wt_" + k, p, f, F32R) for k, (p, f) in {
        "enc1s": (27, 16), "enc2": (16, 288),
        "bott": (32, 576), "dec2a": (64, 288), "dec2b": (32, 288),
        "dec1a": (32, 144), "dec1b": (16, 144), "out": (16, 4),
        "emb": (10, 100)}.items()}
    w_dshT = st("w_dshT", 63, 1, F32R)
    wt_outc = st("wt_outc", 16, 1, F32R)
    bia = {k: st("b_" + k, p, 1) for k, p in {
        "enc1": 16, "enc2": 32, "bott": 64, "dec2": 32, "dec1": 16,
        "out": 4, "dsh": 1, "outc": 1}.items()}
    c025 = st("c025", 100, 1, F32R)
    c1 = st("c1", 100, 2, F32R)
    a2row1 = st("a2row1", 1, NPIX, F32R)
    a2row2 = st("a2row2", 1, NPIX, F32R)
    gmcols = st("gmcols", QCH, NQC)
    lmcols = st("lmcols", QCH, NQC)
    gmT = st("gmT", NQC, QCH, F32R)
    lmT = st("lmT", NQC, QCH, F32R)
    gflat = st("gflat", 1, Q, F32R)
    lflat = st("lflat", 1, Q, F32R)
    out_sb = st("out_sb", 1, 24 * W)

    small = ctx.enter_context(tc.tile_pool(name="small", bufs=8))
    tmp = ctx.enter_context(tc.tile_pool(name="tmp", bufs=2))

    # ------------------------------------------------------------- init
    make_identity(nc, ident[:])

    # inputs / weights (spread over both DMA queues)
    xp3_3 = _r3(xp3[:], 50, 50)
    xt3 = _r3(xt[:], 50, 50)
    dma(xp3_3[:, 1:49, 1:49], d["x3b"].bitcast(F32R))
    for k, nm in [("enc1s", "enc1_wT"), ("enc2", "enc2_wT"),
                  ("bott", "bott_wT"), ("dec2a", "dec2a_wT"),
                  ("dec2b", "dec2b_wT"), ("dec1a", "dec1a_wT"),
                  ("dec1b", "dec1b_wT"), ("out", "out_wT"),
                  ("emb", "emb_wT10")]:
        dma(wt[k][:], d[nm].bitcast(F32R))
    dma(w_dshT[:], d["dsh_wT"].bitcast(F32R))
    dma(wt_outc[:], d["outc_wT"].bitcast(F32R))
    for k, src in [("enc1", "enc1_b"), ("enc2", "enc2_b"), ("bott", "bott_b"),
                   ("dec2", "dec2_b"), ("dec1", "dec1_b"), ("out", "out_b"),
                   ("dsh", "dsh_b"), ("outc", "out_bc")]:
        dma(bia[k][:], d[src].unsqueeze(1))
    dma(_r3(x1p[:], 50, 50)[:, 1:49, 1:49], d["x1c"][None].bitcast(F32R))
    dma(_r3(x2p[:], 50, 50)[:, 1:49, 1:49], d["x2c"][None].bitcast(F32R))
    dma(xt3[6:7, 1:26, 1:49], d["x2c"][None, 0:25, :].bitcast(F32R))

    # border zeroing only (padded row 0 + side columns); interiors are either
    # DMA-filled or written by epilogues, and rows beyond each layer's window
    # are never read.
    def borders(t, pw):
        v = _r3(t[:], pw, pw).bitcast(F32)
        nc.gpsimd.memset(v[:, 0:1, :], 0.0)
        nc.gpsimd.memset(v[:, pw - 1:pw, :], 0.0)
        nc.gpsimd.memset(v[:, 1:pw - 1, 0:1], 0.0)
        nc.gpsimd.memset(v[:, 1:pw - 1, pw - 1:pw], 0.0)

    for t, pw in ((xp3, 50), (x1p, 50), (x2p, 50), (x3cp, 50), (e1p, 50),
                  (p1p, 26), (e2p, 26), (p2p, 14), (btp, 14), (ubp, 26),
                  (d2p, 26), (udp, 50), (xt, 50)):
        borders(t, pw)
    nc.vector.memset(e3x[96:101, :].bitcast(F32), 1.0)   # row 100 = ones
    nc.vector.memset(im2c[:].bitcast(F32), 1.0)
    nc.vector.memset(im2c3[:].bitcast(F32), 1.0)
    nc.gpsimd.memset(c025[:].bitcast(F32), 0.25)
    nc.gpsimd.memset(c1[:].bitcast(F32), 1.0)

    pconv = ctx.enter_context(tc.tile_pool(name="pconv", bufs=2, space="PSUM"))
    pmain = ctx.enter_context(tc.tile_pool(name="pmain", bufs=5, space="PSUM"))
    pmisc = ctx.enter_context(tc.tile_pool(name="pmisc", bufs=1, space="PSUM"))

    # ------------------------------------------------------------ helpers
    def conv9(srcs, cout, row_chunks, w_, func, bias_ap, dst3, scale=1.0):
        """3x3 conv via 9 shifted matmuls accumulating in PSUM.
        srcs: list of (plane3d, wtile, cin)."""
        r0 = 0
        for nr in row_chunks:
            ps = pconv.tile([cout, nr * w_], F32, tag="conv", name="convps")
            ops = []
            for (src3, wtile, cin) in srcs:
                for s in range(9):
                    dy, dx = s // 3, s % 3
                    ops.append((wtile[0:cin, s * cout:(s + 1) * cout],
                                src3[:, r0 + dy:r0 + dy + nr, dx:dx + w_]))
            for i, (l, r) in enumerate(ops):
                nc.tensor.matmul(ps[:], l, r.bitcast(F32R),
                                 start=(i == 0), stop=(i == len(ops) - 1))
            nc.scalar.activation(dst3[:, 1 + r0:1 + r0 + nr, 1:1 + w_],
                                 _r3(ps[:], nr, w_), func,
                                 bias=bias_ap, scale=scale)
            r0 += nr

    def im2col_build(dst, src3, nrows, w_, cin):
        """9 shift-DMAs: dst[s*cin:(s+1)*cin, :] = src3[:, dy:dy+nrows, dx:]"""
        for s in range(9):
            dy, dx = s // 3, s % 3
            dma(dst[s * cin:(s + 1) * cin, 0:nrows * w_],
                src3[:, dy:dy + nrows, dx:dx + w_])

    def conv_im2col(imbufs, cout, row_chunks, w_, func, bias_ap, dst3):
        """conv over pre-built im2col buffers: one matmul per (chunk, kbuf)."""
        r0 = 0
        for nr in row_chunks:
            ps = pconv.tile([cout, nr * w_], F32, tag="conv", name="convps")
            for i, (im, lhsT) in enumerate(imbufs):
                nc.tensor.matmul(ps[:], lhsT, im[:, r0 * w_:(r0 + nr) * w_],
                                 start=(i == 0), stop=(i == len(imbufs) - 1))
            nc.scalar.activation(dst3[:, 1 + r0:1 + r0 + nr, 1:1 + w_],
                                 _r3(ps[:], nr, w_), func, bias=bias_ap)
            r0 += nr

    def pool2(src3, dst3, orows, ocols, cch):
        t1 = tmp.tile([cch, orows * ocols], F32R, tag="pool_a", name="poolt1")
        t2 = tmp.tile([cch, orows * ocols], F32R, tag="pool_b", name="poolt2")
        v = [src3[:, 1 + a:1 + a + 2 * orows:2, 1 + b:1 + b + 2 * ocols:2]
             for a, b in ((0, 0), (1, 1), (0, 1), (1, 0))]
        nc.vector.tensor_max(_r3(t1[:], orows, ocols), v[0], v[1])
        nc.vector.tensor_max(_r3(t2[:], orows, ocols), v[2], v[3])
        nc.vector.tensor_max(dst3[:, 1:1 + orows, 1:1 + ocols],
                             _r3(t1[:], orows, ocols), _r3(t2[:], orows, ocols))

    def up2(src3, dst3, irows, icols):
        s = src3[:, 1:1 + irows, 1:1 + icols]
        for a in (0, 1):
            for b in (0, 1):
                nc.vector.tensor_copy(
                    dst3[:, 1 + a:1 + a + 2 * irows:2,
                         1 + b:1 + b + 2 * icols:2], s)

    def embconv(plane, rows, imbuf, dst, scale):  # noqa
        """1->100 3x3 conv via K=10 im2col matmul (row 9 = ones, wt row 9 =
        bias). Writes dst[0:100, :rows*48] = scale * (conv + emb_b)."""
        n = rows * W
        plane3 = _r3(plane[:], 50, 50)
        for s in range(9):
            dy, dx = s // 3, s % 3
            dma(imbuf[s:s + 1, 0:n], plane3[0:1, dy:dy + rows, dx:dx + W])
        nch = 6 if rows == H else 3
        cw = n // nch
        for ci in range(nch):
            ps = pconv.tile([100, cw], F32, tag="conv", name="convps")
            nc.tensor.matmul(ps[:], wt["emb"][:],
                             imbuf[:, ci * cw:(ci + 1) * cw],
                             start=True, stop=True)
            nc.scalar.activation(dst[0:100, ci * cw:(ci + 1) * cw], ps[:],
                                 AF.Copy, scale=scale)

    def sqrow(ex, rowbuf):
        """|a|^2 row for a ref: square -> per-chunk ones-matmul (M=1) -> ACT
        copies into a [1, 2304] row buffer -> one DMA into ex[100]."""
        nc.scalar.activation(esq[:, 0:NPIX], ex[0:100, 0:NPIX], AF.Square)
        cw = NPIX // 6
        for ci in range(6):
            ps = pconv.tile([1, cw], F32, tag="conv", name="sqps")
            nc.tensor.matmul(ps[:], c025[:],
                             esq[:, ci * cw:(ci + 1) * cw],
                             start=True, stop=True)
            nc.scalar.copy(rowbuf[0:1, ci * cw:(ci + 1) * cw], ps[:])
        dma(ex[100:101, 0:NPIX], rowbuf[0:1, 0:NPIX])

    # ------------------------------------------------------------- U-Net
    e1p3 = _r3(e1p[:], 50, 50)
    p1p3 = _r3(p1p[:], 26, 26)
    e2p3 = _r3(e2p[:], 26, 26)
    p2p3 = _r3(p2p[:], 14, 14)
    btp3 = _r3(btp[:], 14, 14)
    ubp3 = _r3(ubp[:], 26, 26)
    d2p3 = _r3(d2p[:], 26, 26)
    udp3 = _r3(udp[:], 50, 50)
    d1p3 = _r3(d1p[:], 50, 50)
    x3cp3 = _r3(x3cp[:], 50, 50)
    x1p3 = _r3(x1p[:], 50, 50)
    x2p3 = _r3(x2p[:], 50, 50)

    # enc1: im2col (K=27, rows s*3+ci), 39 input rows -> 38 output rows
    im2col_build(im27[:], xp3_3, 39, W, 3)
    conv_im2col([(im27[:], wt["enc1s"][:])], 16, [10, 10, 10, 8], W,
                AF.Relu, bia["enc1"][:], e1p3)
    pool2(e1p3, p1p3, 19, 24, 16)
    conv9([(p1p3, wt["enc2"], 16)], 32, [18], 24, AF.Relu,
          bia["enc2"][:], e2p3)
    pool2(e2p3, p2p3, 9, 12, 32)
    conv9([(p2p3, wt["bott"], 32)], 64, [8], 12, AF.Relu, bia["bott"][:], btp3)
    up2(btp3, ubp3, 8, 12)
    conv9([(ubp3, wt["dec2a"], 64), (e2p3, wt["dec2b"], 32)], 32, [14], 24,
          AF.Relu, bia["dec2"][:], d2p3)
    up2(d2p3, udp3, 14, 24)
    conv9([(udp3, wt["dec1a"], 32), (e1p3, wt["dec1b"], 16)], 16, [10, 10, 6],
          W, AF.Relu, bia["dec1"][:], d1p3)

    # 1x1 output conv -> xt[0:4] (all 4 channels) and x3cp (class-c channel)
    r0 = 0
    for nr in (10, 10, 6):
        rhs = d1p3[:, 1 + r0:1 + r0 + nr, 1:1 + W]
        ps = pconv.tile([4, nr * W], F32, tag="conv", name="convps")
        nc.tensor.matmul(ps[:], wt["out"][:], rhs.bitcast(F32R),
                         start=True, stop=True)
        nc.scalar.activation(xt3[0:4, 1 + r0:1 + r0 + nr, 1:1 + W],
                             _r3(ps[:], nr, W), AF.Identity, bias=bia["out"][:])
        psc = pconv.tile([1, nr * W], F32, tag="conv", name="convps")
        nc.tensor.matmul(psc[:], wt_outc[:], rhs.bitcast(F32R),
                         start=True, stop=True)
        nc.scalar.activation(x3cp3[0:1, 1 + r0:1 + r0 + nr, 1:1 + W],
                             _r3(psc[:], nr, W), AF.Identity,
                             bias=bia["outc"][:])
        r0 += nr

    # ------------------------------------------------- embeddings (filler)
    embconv(x1p, H, im2c[:], e1x[:], -2.0)
    sqrow(e1x[:], a2row1[:])
    embconv(x2p, H, im2c[:], e2x[:], -2.0)
    sqrow(e2x[:], a2row2[:])

    # ------------------------------------------------------- embedding 3
    embconv(x3cp, QROWS, im2c3[:], e3x[:], 1.0)
    nc.scalar.activation(esq[:, 0:Q], e3x[0:100, 0:Q], AF.Square)

    # ------------------------------------------------------- matching
    for c in range(NQC):
        lhsT = e3x[:, c * QCH:(c + 1) * QCH]
        # |b_q|^2 column for this chunk
        b2ps = pmisc.tile([QCH, 2], F32, tag="misc", name="b2ps")
        nc.tensor.matmul(b2ps[:], esq[:, c * QCH:(c + 1) * QCH], c1[:],
                         start=True, stop=True)
        b2c = small.tile([QCH, 1], F32, tag="b2c", name="b2c")
        nc.scalar.copy(b2c[:], b2ps[:, 0:1])
        for r, ex in enumerate((e1x, e2x)):
            mins = small.tile([QCH, len(PCH)], F32, tag="mins", name="mins")
            for j, (p0, pn) in enumerate(PCH):
                ps = pmain.tile([QCH, 512], F32, tag="main", name="mainps")
                nc.tensor.matmul(ps[:, 0:pn], lhsT, ex[:][:, p0:p0 + pn],
                                 start=True, stop=True)
                nc.vector.tensor_reduce(mins[:, j:j + 1], ps[:, 0:pn],
                                        axis=AX.X, op=ALU.min)
            dmin = small.tile([QCH, 1], F32, tag="dmin", name="dmin")
            nc.vector.tensor_reduce(dmin[:], mins[:], axis=AX.X, op=ALU.min)
            dmax = small.tile([QCH, 1], F32, tag="dmax", name="dmax")
            nc.vector.tensor_scalar(dmax[:], dmin[:], b2c[:], 0.0,
                                    op0=ALU.add, op1=ALU.max)
            cols = gmcols if r == 0 else lmcols
            nc.scalar.activation(cols[:, c:c + 1], dmax[:], AF.Tanh, scale=0.5)

    # transpose -> row layout -> xt planes 4 (gm) and 5 (lm)
    for r, (cols, rT, flat) in enumerate(((gmcols, gmT, gflat),
                                          (lmcols, lmT, lflat))):
        ps = pmisc.tile([NQC, QCH], F32, tag="misc", name="miscps")
        nc.tensor.transpose(ps[:], cols[:], ident[:QCH, :QCH])
        nc.scalar.copy(rT[:], ps[:])
        dma(flat[:], rT[:])
        dma(xt3[4 + r:5 + r, 1:26, 1:49], _r3(flat[:], QROWS, W)[:, :, :])

    # ------------------------------------------------------- head conv
    im2col_build(imdsh[:], xt3, 24, W, 7)
    r0 = 0
    for nr in (8, 8, 8):
        ps = pmisc.tile([1, nr * W], F32, tag="misc", name="miscps")
        nc.tensor.matmul(ps[:], w_dshT[:],
                         imdsh[:, r0 * W:(r0 + nr) * W],
                         start=True, stop=True)
        nc.scalar.activation(out_sb[0:1, r0 * W:(r0 + nr) * W],
                             _r3(ps[:], nr, W), AF.Identity,
                             bias=bia["dsh"][:])
        r0 += nr
    nc.sync.dma_start(out_d, out_sb[:])


def build_program():
    import contextlib
    nc = bacc.Bacc("TRN2", target_bir_lowering=False, debug=False,
                   num_devices=8)
    with tile.TileContext(nc) as tc:
        with contextlib.ExitStack() as ctx:
            _emit(nc, tc, ctx)
    nc.compile()
    return nc


def _get_program():
    global _PROG
    if _PROG is None:
        _PROG = build_program()
    return _PROG


CORE_BC = [(0, 2), (0, 3), (1, 2), (1, 3)]
BKEYS = ["enc1_b", "enc2_b", "bott_b", "dec2_b", "dec1_b", "out_b", "dsh_b"]


def _wT_flat(w):
    """[Cout, Cin, 3, 3] -> [Cin, 9*Cout]: col block s holds w[:, :, s//3, s%3].T"""
    cout, cin = w.shape[:2]
    out = np.zeros((cin, 9 * cout), np.float32)
    for s in range(9):
        out[:, s * cout:(s + 1) * cout] = w[:, :, s // 3, s % 3].T
    return out


def _weight_views(inp, flip):
    w = {k: (inp[k][:, :, ::-1, :] if flip else inp[k])
         for k in ["enc1_w", "enc2_w", "bott_w", "dec2_w", "dec1_w",
                   "emb_w", "dsh_w"]}
    m = {}
    # enc1: K-rows ordered s*3+ci to match the im2col build
    m["enc1_wT"] = w["enc1_w"].reshape(16, 3, 9).transpose(2, 1, 0)                               .reshape(27, 16)
    m["enc2_wT"] = _wT_flat(w["enc2_w"])
    m["bott_wT"] = _wT_flat(w["bott_w"])
    m["dec2a_wT"] = _wT_flat(w["dec2_w"][:, :64])
    m["dec2b_wT"] = _wT_flat(w["dec2_w"][:, 64:])
    m["dec1a_wT"] = _wT_flat(w["dec1_w"][:, :32])
    m["dec1b_wT"] = _wT_flat(w["dec1_w"][:, 32:])
    m["out_wT"] = inp["out_w"][:, :, 0, 0].T
    m["emb_wT10"] = np.vstack([w["emb_w"].reshape(100, 9).T,
                               inp["emb_b"][None, :]])
    m["dsh_wT"] = w["dsh_w"].reshape(7, 9).T.reshape(63, 1)
    return m


def make_in_maps(inp):
    wv = [_weight_views(inp, False), _weight_views(inp, True)]
    maps = []
    for k8 in range(8):
        n_idx, half = k8 // 2, k8 % 2
        b, c = CORE_BC[n_idx]
        m = dict(wv[half])
        x1c, x2c, x3b = inp["x1"][b, c], inp["x2"][b, c], inp["x3"][b]
        if half:
            x1c, x2c, x3b = x1c[::-1], x2c[::-1], x3b[:, ::-1]
        m["x1c"], m["x2c"], m["x3b"] = x1c, x2c, x3b
        for k in BKEYS:
            m[k] = inp[k]
        m["outc_wT"] = inp["out_w"][c, :, 0, 0][:, None]
        m["out_bc"] = inp["out_b"][c:c + 1]
        maps.append({k: np.ascontiguousarray(v, np.float32)
                     for k, v in m.items()})
    return maps


def assemble(results):
    out = np.zeros((2, 2, H, W), np.float32)
    for k8, r in enumerate(results):
        n_idx, half = k8 // 2, k8 % 2
        b, c = CORE_BC[n_idx]
        y = r["out"]
        if half == 0:
            out[b, c - 2, 0:24] = y
        else:
            out[b, c - 2, 24:48] = y[::-1]
    return out


def kernel(**inputs):
    inp = {k: np.asarray(v) for k, v in inputs.items()}
    nc = _get_program()
    maps = make_in_maps(inp)
    res = run_bass_kernel_spmd(nc, maps, core_ids=list(range(8)), trace=False)
    return assemble(res.results)


# revision 22
# speedup vs baseline: 1.0307x; 1.0307x over previous
"""FEELVOS fused kernel for TRN2, 8-core SPMD.

Sharding: the reference only returns logits for classes C-2, C-1, so only 4 of
the 8 fused (batch, class) items matter. 8 cores = 4 (b, c) pairs x 2 frame
halves (top/bottom 24 rows). Bottom-half cores receive row-flipped inputs and
row-flipped conv kernels so every core runs the identical program computing
"top 25 rows" of its (possibly flipped) frame; the host un-flips on gather.

Per core:
  - partial U-Net on x3[b] (top-aligned row windows; convs as im2col or
    9-shifted matmuls on zero-padded [C, 50x50] SBUF planes, fp32r)
  - 100-d embeddings of x1[b,c], x2[b,c] (full frame, ref side) and of the
    class-c channel of the U-Net output (25-row window, query side)
  - distance matrix via one K=101 matmul per tile: rows 0..99 carry
    e3 . (-2 e_ref), row 100 carries 1 * |a_p|^2; DVE reduce_min over the
    free axis; |b_q|^2 added after the min (per-chunk column matmul);
    then tanh(d/2) == 1 - 2*sigmoid(-d)
  - 3x3 head conv (im2col, K=63) over [x3s(4ch), gm, lm, x2] -> 24 rows.
"""
import numpy as np

import concourse.bass as bass
import concourse.bacc as bacc
import concourse.tile as tile
from concourse import mybir
from concourse.bass_utils import run_bass_kernel_spmd
from concourse.masks import make_identity

F32 = mybir.dt.float32
F32R = mybir.dt.float32r
AF = mybir.ActivationFunctionType
ALU = mybir.AluOpType
AX = mybir.AxisListType

H = W = 48
NPIX = H * W                 # 2304 ref pixels
QROWS = 25
Q = QROWS * W                # 1200 query pixels
QCH, NQC = 120, 10           # query chunking for the distance matmul
PCH = [(0, 512), (512, 512), (1024, 512), (1536, 512), (2048, 256)]

_PROG = None


def _r3(ap, h, w):
    return ap.rearrange("c (h w) -> c h w", h=h, w=w)



# blobW column layout: (name, rows, cols); all segments base partition 0.
WSEGS = [("enc1s", 27, 16), ("enc2", 16, 288), ("bott", 32, 576),
         ("dec2a", 64, 288), ("dec2b", 32, 288), ("dec1a", 32, 144),
         ("dec1b", 16, 144), ("out", 16, 4), ("emb", 10, 100),
         ("dsh", 63, 1), ("outc", 16, 1),
         ("b_enc1", 16, 1), ("b_enc2", 32, 1), ("b_bott", 64, 1),
         ("b_dec2", 32, 1), ("b_dec1", 16, 1), ("b_out", 4, 1),
         ("b_dsh", 1, 1), ("b_outc", 1, 1)]
WOFF = {}
_o = 0
for _nm, _r, _c in WSEGS:
    WOFF[_nm] = _o
    _o += _c
WCOLS = _o


def _emit(nc, tc, ctx):
    # ------------------------------------------------------------- dram io
    bW = nc.dram_tensor("blobW", [64, WCOLS], F32, kind="ExternalInput").ap()
    bX = nc.dram_tensor("blobX", [3, 7500], F32, kind="ExternalInput").ap()
    out_d = nc.dram_tensor("out", [24, W], F32, kind="ExternalOutput").ap()

    # round-robin DMA dispatch over the two HWDGE engines
    _dmaq = [nc.sync, nc.scalar]
    _qi = [0]

    def dma(out, in_):
        eng = _dmaq[_qi[0] % len(_dmaq)]
        _qi[0] += 1
        eng.dma_start(out, in_)

    # ------------------------------------------------------------- sbuf
    sb = ctx.enter_context(tc.tile_pool(name="sb", bufs=1))

    def st(name, p, f, dt=F32):
        return sb.tile([p, f], dt, tag=name, name=name)

    blobw = st("blobw", 64, WCOLS, F32R)
    blobx = st("blobx", 3, 7500, F32R)

    def wseg(nm):
        rows, cols = next((r, c) for n, r, c in WSEGS if n == nm)
        return blobw[0:rows, WOFF[nm]:WOFF[nm] + cols]

    wt = {nm: wseg(nm) for nm, _, _ in WSEGS if not nm.startswith("b_")}
    bia = {nm[2:]: wseg(nm).bitcast(F32)
           for nm, _, _ in WSEGS if nm.startswith("b_")}

    xp3 = blobx[0:3, 0:2500]
    x1p = blobx[0:1, 2500:5000]
    x2p = blobx[0:1, 5000:7500]

    # device-written padded planes
    x3cp = st("x3cp", 1, 2500, F32R)
    e1p = st("e1p", 16, 2500, F32R)
    p1p = st("p1p", 16, 676, F32R)
    e2p = st("e2p", 32, 676, F32R)
    p2p = st("p2p", 32, 196, F32R)
    btp = st("btp", 64, 196, F32R)
    ubp = st("ubp", 64, 676, F32R)
    d2p = st("d2p", 32, 676, F32R)
    udp = st("udp", 32, 2500, F32R)
    d1p = st("d1p", 16, 2500, F32R)
    xt = st("xt", 7, 2500, F32R)

    im2c = st("im2c", 10, NPIX, F32R)       # emb im2col (e1/e2)
    im2c3 = st("im2c3", 10, Q, F32R)        # emb im2col (e3)
    im27 = st("im27", 27, 39 * W, F32R)     # enc1 im2col (rows s*3+ci)
    imdsh = st("imdsh", 63, 24 * W, F32R)   # head im2col (rows s*7+ci)
    e1x = st("e1x", 101, NPIX, F32R)
    e2x = st("e2x", 101, NPIX, F32R)
    e3x = st("e3x", 101, Q, F32R)
    esq = st("esq", 100, NPIX, F32R)
    ident = st("ident", 128, 128)

    c025 = st("c025", 100, 1, F32R)
    c1 = st("c1", 100, 2, F32R)
    a2row1 = st("a2row1", 1, NPIX, F32R)
    a2row2 = st("a2row2", 1, NPIX, F32R)
    gflat = st("gflat", 1, Q, F32R)
    lflat = st("lflat", 1, Q, F32R)
    out_sb = st("out_sb", 1, 24 * W)

    small = ctx.enter_context(tc.tile_pool(name="small", bufs=8))
    tmp = ctx.enter_context(tc.tile_pool(name="tmp", bufs=2))

    # ------------------------------------------------------------- init
    dma(blobx[:], bX.bitcast(F32R))
    dma(blobw[:], bW.bitcast(F32R))
    make_identity(nc, ident[:])

    xp3_3 = _r3(xp3, 50, 50)
    xt3 = _r3(xt[:], 50, 50)
    x3cp3 = _r3(x3cp[:], 50, 50)
    x1p3 = _r3(x1p, 50, 50)
    x2p3 = _r3(x2p, 50, 50)

    # borders of device-written planes; xt fully (ch 4/5 are read by the
    # early imdsh build before gm/lm land, then patched)
    def borders(t, pw):
        v = _r3(t[:], pw, pw).bitcast(F32)
        nc.gpsimd.memset(v[:, 0:1, :], 0.0)
        nc.gpsimd.memset(v[:, pw - 1:pw, :], 0.0)
        nc.gpsimd.memset(v[:, 1:pw - 1, 0:1], 0.0)
        nc.gpsimd.memset(v[:, 1:pw - 1, pw - 1:pw], 0.0)

    nc.gpsimd.memset(xt[:].bitcast(F32), 0.0)
    for t, pw in ((x3cp, 50), (e1p, 50), (p1p, 26), (e2p, 26), (p2p, 14),
                  (btp, 14), (ubp, 26), (d2p, 26), (udp, 50)):
        borders(t, pw)
    nc.vector.memset(e3x[96:101, :].bitcast(F32), 1.0)   # row 100 = ones
    nc.vector.memset(im2c[:].bitcast(F32), 1.0)
    nc.vector.memset(im2c3[:].bitcast(F32), 1.0)
    nc.gpsimd.memset(c025[:].bitcast(F32), 0.25)
    nc.gpsimd.memset(c1[:].bitcast(F32), 1.0)
    # xt channel 6 = x2 rows 0..24 (from the pre-padded x2 plane)
    dma(xt3[6:7, 1:26, 1:49], x2p3[0:1, 1:26, 1:49])

    pconv = ctx.enter_context(tc.tile_pool(name="pconv", bufs=2, space="PSUM"))
    pmain = ctx.enter_context(tc.tile_pool(name="pmain", bufs=2, space="PSUM"))

    # ------------------------------------------------------------ helpers
    def conv9(srcs, cout, row_chunks, w_, func, bias_ap, dst3, scale=1.0):
        """3x3 conv via 9 shifted matmuls accumulating in PSUM.
        srcs: list of (plane3d, wtile, cin)."""
        r0 = 0
        for nr in row_chunks:
            ps = pconv.tile([cout, nr * w_], F32, tag="conv", name="convps")
            ops = []
            for (src3, wtile, cin) in srcs:
                for s in range(9):
                    dy, dx = s // 3, s % 3
                    ops.append((wtile[0:cin, s * cout:(s + 1) * cout],
                                src3[:, r0 + dy:r0 + dy + nr, dx:dx + w_]))
            for i, (l, r) in enumerate(ops):
                nc.tensor.matmul(ps[:], l, r.bitcast(F32R),
                                 start=(i == 0), stop=(i == len(ops) - 1))
            nc.scalar.activation(dst3[:, 1 + r0:1 + r0 + nr, 1:1 + w_],
                                 _r3(ps[:], nr, w_), func,
                                 bias=bias_ap, scale=scale)
            r0 += nr

    def im2col_build(dst, src3, nrows, w_, cin, chans=None):
        """9 shift-DMAs: dst[s*cin+c0 : s*cin+c1] = src3[c0:c1, shifted]"""
        c0, c1_ = (0, cin) if chans is None else chans
        for s in range(9):
            dy, dx = s // 3, s % 3
            dma(dst[s * cin + c0:s * cin + c1_, 0:nrows * w_],
                src3[c0:c1_, dy:dy + nrows, dx:dx + w_])

    def conv_im2col(imbufs, cout, row_chunks, w_, func, bias_ap, dst3):
        r0 = 0
        for nr in row_chunks:
            ps = pconv.tile([cout, nr * w_], F32, tag="conv", name="convps")
            for i, (im, lhsT) in enumerate(imbufs):
                nc.tensor.matmul(ps[:], lhsT, im[:, r0 * w_:(r0 + nr) * w_],
                                 start=(i == 0), stop=(i == len(imbufs) - 1))
            nc.scalar.activation(dst3[:, 1 + r0:1 + r0 + nr, 1:1 + w_],
                                 _r3(ps[:], nr, w_), func, bias=bias_ap)
            r0 += nr

    def pool2(src3, dst3, orows, ocols, cch):
        t1 = tmp.tile([cch, orows * ocols], F32R, tag="pool_a", name="poolt1")
        t2 = tmp.tile([cch, orows * ocols], F32R, tag="pool_b", name="poolt2")
        v = [src3[:, 1 + a:1 + a + 2 * orows:2, 1 + b:1 + b + 2 * ocols:2]
             for a, b in ((0, 0), (1, 1), (0, 1), (1, 0))]
        nc.vector.tensor_max(_r3(t1[:], orows, ocols), v[0], v[1])
        nc.vector.tensor_max(_r3(t2[:], orows, ocols), v[2], v[3])
        nc.vector.tensor_max(dst3[:, 1:1 + orows, 1:1 + ocols],
                             _r3(t1[:], orows, ocols), _r3(t2[:], orows, ocols))

    def up2(src3, dst3, irows, icols):
        s = src3[:, 1:1 + irows, 1:1 + icols]
        for a in (0, 1):
            for b in (0, 1):
                nc.vector.tensor_copy(
                    dst3[:, 1 + a:1 + a + 2 * irows:2,
                         1 + b:1 + b + 2 * icols:2], s)

    def embconv(plane3, rows, imbuf, dst, scale):
        """1->100 3x3 conv via K=10 im2col matmul (row 9 = ones, wt row 9 =
        emb bias)."""
        n = rows * W
        for s in range(9):
            dy, dx = s // 3, s % 3
            dma(imbuf[s:s + 1, 0:n], plane3[0:1, dy:dy + rows, dx:dx + W])
        nch = 6 if rows == H else 3
        cw = n // nch
        for ci in range(nch):
            ps = pconv.tile([100, cw], F32, tag="conv", name="convps")
            nc.tensor.matmul(ps[:], wt["emb"],
                             imbuf[:, ci * cw:(ci + 1) * cw],
                             start=True, stop=True)
            nc.scalar.activation(dst[0:100, ci * cw:(ci + 1) * cw], ps[:],
                                 AF.Copy, scale=scale)

    def sqrow(ex, rowbuf):
        nc.scalar.activation(esq[:, 0:NPIX], ex[0:100, 0:NPIX], AF.Square)
        cw = NPIX // 6
        for ci in range(6):
            ps = pconv.tile([1, cw], F32, tag="conv", name="sqps")
            nc.tensor.matmul(ps[:], c025[:],
                             esq[:, ci * cw:(ci + 1) * cw],
                             start=True, stop=True)
            nc.scalar.copy(rowbuf[0:1, ci * cw:(ci + 1) * cw], ps[:])
        dma(ex[100:101, 0:NPIX], rowbuf[0:1, 0:NPIX])

    # ------------------------------------------------------------- U-Net
    e1p3 = _r3(e1p[:], 50, 50)
    p1p3 = _r3(p1p[:], 26, 26)
    e2p3 = _r3(e2p[:], 26, 26)
    p2p3 = _r3(p2p[:], 14, 14)
    btp3 = _r3(btp[:], 14, 14)
    ubp3 = _r3(ubp[:], 26, 26)
    d2p3 = _r3(d2p[:], 26, 26)
    udp3 = _r3(udp[:], 50, 50)
    d1p3 = _r3(d1p[:], 50, 50)

    im2col_build(im27[:], xp3_3, 39, W, 3)
    conv_im2col([(im27[:], wt["enc1s"])], 16, [10, 10, 10, 8], W,
                AF.Relu, bia["enc1"], e1p3)
    pool2(e1p3, p1p3, 19, 24, 16)
    conv9([(p1p3, wt["enc2"], 16)], 32, [18], 24, AF.Relu, bia["enc2"], e2p3)
    pool2(e2p3, p2p3, 9, 12, 32)
    conv9([(p2p3, wt["bott"], 32)], 64, [8], 12, AF.Relu, bia["bott"], btp3)
    up2(btp3, ubp3, 8, 12)
    conv9([(ubp3, wt["dec2a"], 64), (e2p3, wt["dec2b"], 32)], 32, [14], 24,
          AF.Relu, bia["dec2"], d2p3)
    up2(d2p3, udp3, 14, 24)
    conv9([(udp3, wt["dec1a"], 32), (e1p3, wt["dec1b"], 16)], 16, [10, 10, 6],
          W, AF.Relu, bia["dec1"], d1p3)

    # 1x1 output conv -> xt[0:4] (all 4 channels) and x3cp (class-c channel)
    r0 = 0
    for nr in (10, 10, 6):
        rhs = d1p3[:, 1 + r0:1 + r0 + nr, 1:1 + W]
        ps = pconv.tile([4, nr * W], F32, tag="conv", name="convps")
        nc.tensor.matmul(ps[:], wt["out"], rhs.bitcast(F32R),
                         start=True, stop=True)
        nc.scalar.activation(xt3[0:4, 1 + r0:1 + r0 + nr, 1:1 + W],
                             _r3(ps[:], nr, W), AF.Identity, bias=bia["out"])
        psc = pconv.tile([1, nr * W], F32, tag="conv", name="convps")
        nc.tensor.matmul(psc[:], wt["outc"], rhs.bitcast(F32R),
                         start=True, stop=True)
        nc.scalar.activation(x3cp3[0:1, 1 + r0:1 + r0 + nr, 1:1 + W],
                             _r3(psc[:], nr, W), AF.Identity,
                             bias=bia["outc"])
        r0 += nr

    # early head-conv im2col (channels 4/5 hold stale zeros; patched later)
    im2col_build(imdsh[:], xt3, 24, W, 7)

    # ------------------------------------------------- embeddings (filler)
    embconv(x1p3, H, im2c[:], e1x[:], -2.0)
    sqrow(e1x[:], a2row1[:])
    embconv(x2p3, H, im2c[:], e2x[:], -2.0)
    sqrow(e2x[:], a2row2[:])

    # ------------------------------------------------------- embedding 3
    embconv(x3cp3, QROWS, im2c3[:], e3x[:], 1.0)
    nc.scalar.activation(esq[:, 0:Q], e3x[0:100, 0:Q], AF.Square)

    # ------------------------------------------------------- matching
    # P chunks: two [120,1024] psum tiles (2 matmuls each) + one [120,256]
    for c in range(NQC):
        lhsT = e3x[:, c * QCH:(c + 1) * QCH]
        b2ps = pconv.tile([QCH, 2], F32, tag="conv", name="b2ps")
        nc.tensor.matmul(b2ps[:], esq[:, c * QCH:(c + 1) * QCH], c1[:],
                         start=True, stop=True)
        b2c = small.tile([QCH, 1], F32, tag="b2c", name="b2c")
        nc.scalar.copy(b2c[:], b2ps[:, 0:1])
        for r, ex in enumerate((e1x, e2x)):
            mins = small.tile([QCH, 3], F32, tag="mins", name="mins")
            for j in range(2):
                ps = pmain.tile([QCH, 1024], F32, tag="main", name="mainps")
                nc.tensor.matmul(ps[:, 0:512], lhsT,
                                 ex[:][:, j * 1024:j * 1024 + 512],
                                 start=True, stop=True)
                nc.tensor.matmul(ps[:, 512:1024], lhsT,
                                 ex[:][:, j * 1024 + 512:(j + 1) * 1024],
                                 start=True, stop=True)
                nc.vector.tensor_reduce(mins[:, j:j + 1], ps[:],
                                        axis=AX.X, op=ALU.min)
            pt = pmain.tile([QCH, 256], F32, tag="maint", name="maintps", bufs=2)
            nc.tensor.matmul(pt[:], lhsT, ex[:][:, 2048:2304],
                             start=True, stop=True)
            nc.vector.tensor_reduce(mins[:, 2:3], pt[:], axis=AX.X,
                                    op=ALU.min)
            dmin = small.tile([QCH, 1], F32, tag="dmin", name="dmin")
            nc.vector.tensor_reduce(dmin[:], mins[:], axis=AX.X, op=ALU.min)
            dmax = small.tile([QCH, 1], F32, tag="dmax", name="dmax")
            nc.vector.tensor_scalar(dmax[:], dmin[:], b2c[:], 0.0,
                                    op0=ALU.add, op1=ALU.max)
            gcol = small.tile([QCH, 1], F32, tag="gcol", name="gcol")
            nc.scalar.activation(gcol[:], dmax[:], AF.Tanh, scale=0.5)
            # transpose this chunk's column into the flat row now
            pst = pconv.tile([1, QCH], F32, tag="conv", name="gmtps")
            nc.tensor.transpose(pst[:], gcol[:], ident[:QCH, :QCH])
            flat = gflat if r == 0 else lflat
            nc.scalar.copy(flat[0:1, c * QCH:(c + 1) * QCH], pst[:])

    # gm/lm rows -> xt planes 4/5, patch imdsh channels 4/5, head conv
    dma(xt3[4:5, 1:26, 1:49], _r3(gflat[:], QROWS, W)[:, :, :])
    dma(xt3[5:6, 1:26, 1:49], _r3(lflat[:], QROWS, W)[:, :, :])
    im2col_build(imdsh[:], xt3, 24, W, 7, chans=(4, 6))
    r0 = 0
    for nr in (8, 8, 8):
        ps = pconv.tile([1, nr * W], F32, tag="conv", name="convps")
        nc.tensor.matmul(ps[:], wt["dsh"],
                         imdsh[:, r0 * W:(r0 + nr) * W],
                         start=True, stop=True)
        nc.scalar.activation(out_sb[0:1, r0 * W:(r0 + nr) * W],
                             _r3(ps[:], nr, W), AF.Identity,
                             bias=bia["dsh"])
        r0 += nr
    nc.sync.dma_start(out_d, out_sb[:])


def build_program():
    import contextlib
    nc = bacc.Bacc("TRN2", target_bir_lowering=False, debug=False,
                   num_devices=8)
    with tile.TileContext(nc) as tc:
        with contextlib.ExitStack() as ctx:
            _emit(nc, tc, ctx)
    nc.compile()
    return nc


def _get_program():
    global _PROG
    if _PROG is None:
        _PROG = build_program()
    return _PROG


CORE_BC = [(0, 2), (0, 3), (1, 2), (1, 3)]


def _wT_flat(w):
    """[Cout, Cin, 3, 3] -> [Cin, 9*Cout]: col block s holds w[:, :, s//3, s%3].T"""
    cout, cin = w.shape[:2]
    out = np.zeros((cin, 9 * cout), np.float32)
    for s in range(9):
        out[:, s * cout:(s + 1) * cout] = w[:, :, s // 3, s % 3].T
    return out


def _pad50(img):
    out = np.zeros((50, 50), np.float32)
    out[1:49, 1:49] = img
    return out


def _blobw(inp, flip, c):
    w = {k: (inp[k][:, :, ::-1, :] if flip else inp[k])
         for k in ["enc1_w", "enc2_w", "bott_w", "dec2_w", "dec1_w",
                   "emb_w", "dsh_w"]}
    seg = {}
    seg["enc1s"] = w["enc1_w"].reshape(16, 3, 9).transpose(2, 1, 0) \
                              .reshape(27, 16)
    seg["enc2"] = _wT_flat(w["enc2_w"])
    seg["bott"] = _wT_flat(w["bott_w"])
    seg["dec2a"] = _wT_flat(w["dec2_w"][:, :64])
    seg["dec2b"] = _wT_flat(w["dec2_w"][:, 64:])
    seg["dec1a"] = _wT_flat(w["dec1_w"][:, :32])
    seg["dec1b"] = _wT_flat(w["dec1_w"][:, 32:])
    seg["out"] = inp["out_w"][:, :, 0, 0].T
    seg["emb"] = np.vstack([w["emb_w"].reshape(100, 9).T,
                            inp["emb_b"][None, :]])
    seg["dsh"] = w["dsh_w"].reshape(7, 9).T.reshape(63, 1)
    seg["outc"] = inp["out_w"][c, :, 0, 0][:, None]
    for k in ["enc1", "enc2", "bott", "dec2", "dec1", "out", "dsh"]:
        seg["b_" + k] = inp[k + "_b"][:, None]
    seg["b_outc"] = inp["out_b"][c:c + 1][:, None]
    blob = np.zeros((64, WCOLS), np.float32)
    for nm, rows, cols in WSEGS:
        blob[0:rows, WOFF[nm]:WOFF[nm] + cols] = seg[nm]
    return blob


def make_in_maps(inp):
    maps = []
    for k8 in range(8):
        n_idx, half = k8 // 2, k8 % 2
        b, c = CORE_BC[n_idx]
        x1c, x2c, x3b = inp["x1"][b, c], inp["x2"][b, c], inp["x3"][b]
        if half:
            x1c, x2c, x3b = x1c[::-1], x2c[::-1], x3b[:, ::-1]
        blobx = np.zeros((3, 7500), np.float32)
        for ch in range(3):
            blobx[ch, 0:2500] = _pad50(x3b[ch]).ravel()
        blobx[0, 2500:5000] = _pad50(x1c).ravel()
        blobx[0, 5000:7500] = _pad50(x2c).ravel()
        maps.append({"blobW": np.ascontiguousarray(_blobw(inp, bool(half), c)),
                     "blobX": np.ascontiguousarray(blobx)})
    return maps


def assemble(results):
    out = np.zeros((2, 2, H, W), np.float32)
    for k8, r in enumerate(results):
        n_idx, half = k8 // 2, k8 % 2
        b, c = CORE_BC[n_idx]
        y = r["out"]
        if half == 0:
            out[b, c - 2, 0:24] = y
        else:
            out[b, c - 2, 24:48] = y[::-1]
    return out


def kernel(**inputs):
    inp = {k: np.asarray(v) for k, v in inputs.items()}
    nc = _get_program()
    maps = make_in_maps(inp)
    res = run_bass_kernel_spmd(nc, maps, core_ids=list(range(8)), trace=False)
    return assemble(res.results)


# revision 23
# speedup vs baseline: 1.1276x; 1.0940x over previous
"""FEELVOS fused kernel for TRN2, 8-core SPMD.

Sharding: the reference only returns logits for classes C-2, C-1, so only 4 of
the 8 fused (batch, class) items matter. 8 cores = 4 (b, c) pairs x 2 frame
halves (top/bottom 24 rows). Bottom-half cores receive row-flipped inputs and
row-flipped conv kernels so every core runs the identical program computing
"top 25 rows" of its (possibly flipped) frame; the host un-flips on gather.

Per core:
  - partial U-Net on x3[b] (top-aligned row windows; convs as im2col or
    9-shifted matmuls on zero-padded [C, 50x50] SBUF planes, fp32r)
  - 100-d embeddings of x1[b,c], x2[b,c] (full frame, ref side) and of the
    class-c channel of the U-Net output (25-row window, query side)
  - distance matrix via one K=101 matmul per tile: rows 0..99 carry
    e3 . (-2 e_ref), row 100 carries 1 * |a_p|^2; DVE reduce_min over the
    free axis; |b_q|^2 added after the min (per-chunk column matmul);
    then tanh(d/2) == 1 - 2*sigmoid(-d)
  - 3x3 head conv (im2col, K=63) over [x3s(4ch), gm, lm, x2] -> 24 rows.
"""
import numpy as np

import concourse.bass as bass
import concourse.bacc as bacc
import concourse.tile as tile
from concourse import mybir
from concourse.bass_utils import run_bass_kernel_spmd
from concourse.masks import make_identity

F32 = mybir.dt.float32
F32R = mybir.dt.float32r
AF = mybir.ActivationFunctionType
ALU = mybir.AluOpType
AX = mybir.AxisListType

H = W = 48
NPIX = H * W                 # 2304 ref pixels
QROWS = 25
Q = QROWS * W                # 1200 query pixels
QCH, NQC = 120, 10           # query chunking for the distance matmul
PCH = [(0, 512), (512, 512), (1024, 512), (1536, 512), (2048, 256)]

_PROG = None


def _r3(ap, h, w):
    return ap.rearrange("c (h w) -> c h w", h=h, w=w)



# blobW column layout: (name, rows, cols); all segments base partition 0.
WSEGS = [("enc1s", 27, 16), ("enc2", 16, 288), ("bott", 32, 576),
         ("dec2a", 64, 288), ("dec2b", 32, 288), ("dec1a", 32, 144),
         ("dec1b", 16, 144), ("out", 16, 4), ("emb", 10, 100),
         ("dsh", 63, 1), ("outc", 16, 1),
         ("b_enc1", 16, 1), ("b_enc2", 32, 1), ("b_bott", 64, 1),
         ("b_dec2", 32, 1), ("b_dec1", 16, 1), ("b_out", 4, 1),
         ("b_dsh", 1, 1), ("b_outc", 1, 1)]
WOFF = {}
_o = 0
for _nm, _r, _c in WSEGS:
    WOFF[_nm] = _o
    _o += _c
WCOLS = _o


def _emit(nc, tc, ctx):
    # ------------------------------------------------------------- dram io
    bW = nc.dram_tensor("blobW", [64, WCOLS], F32, kind="ExternalInput").ap()
    bX1 = nc.dram_tensor("blobX1", [27, 39 * W], F32,
                         kind="ExternalInput").ap()
    bX2 = nc.dram_tensor("blobX2", [10, 2 * NPIX], F32,
                         kind="ExternalInput").ap()
    bX3 = nc.dram_tensor("blobX3", [1, Q], F32, kind="ExternalInput").ap()
    out_d = nc.dram_tensor("out", [24, W], F32, kind="ExternalOutput").ap()

    # round-robin DMA dispatch over the two HWDGE engines
    _dmaq = [nc.sync, nc.scalar]
    _qi = [0]

    def dma(out, in_):
        eng = _dmaq[_qi[0] % len(_dmaq)]
        _qi[0] += 1
        eng.dma_start(out, in_)

    # ------------------------------------------------------------- sbuf
    sb = ctx.enter_context(tc.tile_pool(name="sb", bufs=1))

    def st(name, p, f, dt=F32):
        return sb.tile([p, f], dt, tag=name, name=name)

    blobw = st("blobw", 64, WCOLS, F32R)
    im27 = st("im27", 27, 39 * W, F32R)     # enc1 im2col (host-built)
    im2c12 = st("im2c12", 10, 2 * NPIX, F32R)  # emb im2col e1|e2 (host-built)
    xt6row = st("xt6row", 1, Q, F32R)

    def wseg(nm):
        rows, cols = next((r, c) for n, r, c in WSEGS if n == nm)
        return blobw[0:rows, WOFF[nm]:WOFF[nm] + cols]

    wt = {nm: wseg(nm) for nm, _, _ in WSEGS if not nm.startswith("b_")}
    bia = {nm[2:]: wseg(nm).bitcast(F32)
           for nm, _, _ in WSEGS if nm.startswith("b_")}

    # device-written padded planes
    x3cp = st("x3cp", 1, 2500, F32R)
    e1p = st("e1p", 16, 2500, F32R)
    p1p = st("p1p", 16, 676, F32R)
    e2p = st("e2p", 32, 676, F32R)
    p2p = st("p2p", 32, 196, F32R)
    btp = st("btp", 64, 196, F32R)
    ubp = st("ubp", 64, 676, F32R)
    d2p = st("d2p", 32, 676, F32R)
    udp = st("udp", 32, 2500, F32R)
    d1p = st("d1p", 16, 2500, F32R)
    xt = st("xt", 7, 2500, F32R)

    im2c3 = st("im2c3", 10, Q, F32R)        # emb im2col (e3)
    imdsh = st("imdsh", 63, 24 * W, F32R)   # head im2col (rows s*7+ci)
    e1x = st("e1x", 101, NPIX, F32R)
    e2x = st("e2x", 101, NPIX, F32R)
    e3x = st("e3x", 101, Q, F32R)
    esq = st("esq", 100, NPIX, F32R)
    ident = st("ident", 128, 128)

    c025 = st("c025", 100, 1, F32R)
    c1 = st("c1", 100, 2, F32R)
    a2row1 = st("a2row1", 1, NPIX, F32R)
    a2row2 = st("a2row2", 1, NPIX, F32R)
    gflat = st("gflat", 1, Q, F32R)
    lflat = st("lflat", 1, Q, F32R)
    out_sb = st("out_sb", 1, 24 * W)

    small = ctx.enter_context(tc.tile_pool(name="small", bufs=8))
    tmp = ctx.enter_context(tc.tile_pool(name="tmp", bufs=2))

    # ------------------------------------------------------------- init
    dma(im27[:], bX1.bitcast(F32R))
    dma(blobw[:], bW.bitcast(F32R))
    dma(im2c12[:], bX2.bitcast(F32R))
    dma(xt6row[:], bX3.bitcast(F32R))
    make_identity(nc, ident[:])

    xt3 = _r3(xt[:], 50, 50)
    x3cp3 = _r3(x3cp[:], 50, 50)

    # borders of device-written planes; xt fully (ch 4/5 are read by the
    # early imdsh build before gm/lm land, then patched)
    def borders(t, pw):
        v = _r3(t[:], pw, pw).bitcast(F32)
        nc.gpsimd.memset(v[:, 0:1, :], 0.0)
        nc.gpsimd.memset(v[:, pw - 1:pw, :], 0.0)
        nc.gpsimd.memset(v[:, 1:pw - 1, 0:1], 0.0)
        nc.gpsimd.memset(v[:, 1:pw - 1, pw - 1:pw], 0.0)

    nc.gpsimd.memset(xt[:].bitcast(F32), 0.0)
    for t, pw in ((x3cp, 50), (e1p, 50), (p1p, 26), (e2p, 26), (p2p, 14),
                  (btp, 14), (ubp, 26), (d2p, 26), (udp, 50)):
        borders(t, pw)
    nc.vector.memset(e3x[96:101, :].bitcast(F32), 1.0)   # row 100 = ones
    nc.vector.memset(im2c3[:].bitcast(F32), 1.0)
    nc.gpsimd.memset(c025[:].bitcast(F32), 0.25)
    nc.gpsimd.memset(c1[:].bitcast(F32), 1.0)
    # xt channel 6 = x2 rows 0..24
    dma(xt3[6:7, 1:26, 1:49], xt6row[:])

    pconv = ctx.enter_context(tc.tile_pool(name="pconv", bufs=2, space="PSUM"))
    pmain = ctx.enter_context(tc.tile_pool(name="pmain", bufs=2, space="PSUM"))

    # ------------------------------------------------------------ helpers
    def conv9(srcs, cout, row_chunks, w_, func, bias_ap, dst3, scale=1.0):
        """3x3 conv via 9 shifted matmuls accumulating in PSUM.
        srcs: list of (plane3d, wtile, cin)."""
        r0 = 0
        for nr in row_chunks:
            ps = pconv.tile([cout, nr * w_], F32, tag="conv", name="convps")
            ops = []
            for (src3, wtile, cin) in srcs:
                for s in range(9):
                    dy, dx = s // 3, s % 3
                    ops.append((wtile[0:cin, s * cout:(s + 1) * cout],
                                src3[:, r0 + dy:r0 + dy + nr, dx:dx + w_]))
            for i, (l, r) in enumerate(ops):
                nc.tensor.matmul(ps[:], l, r.bitcast(F32R),
                                 start=(i == 0), stop=(i == len(ops) - 1))
            nc.scalar.activation(dst3[:, 1 + r0:1 + r0 + nr, 1:1 + w_],
                                 _r3(ps[:], nr, w_), func,
                                 bias=bias_ap, scale=scale)
            r0 += nr

    def im2col_build(dst, src3, nrows, w_, cin, chans=None):
        """9 shift-DMAs: dst[s*cin+c0 : s*cin+c1] = src3[c0:c1, shifted]"""
        c0, c1_ = (0, cin) if chans is None else chans
        for s in range(9):
            dy, dx = s // 3, s % 3
            dma(dst[s * cin + c0:s * cin + c1_, 0:nrows * w_],
                src3[c0:c1_, dy:dy + nrows, dx:dx + w_])

    def conv_im2col(imbufs, cout, row_chunks, w_, func, bias_ap, dst3):
        r0 = 0
        for nr in row_chunks:
            ps = pconv.tile([cout, nr * w_], F32, tag="conv", name="convps")
            for i, (im, lhsT) in enumerate(imbufs):
                nc.tensor.matmul(ps[:], lhsT, im[:, r0 * w_:(r0 + nr) * w_],
                                 start=(i == 0), stop=(i == len(imbufs) - 1))
            nc.scalar.activation(dst3[:, 1 + r0:1 + r0 + nr, 1:1 + w_],
                                 _r3(ps[:], nr, w_), func, bias=bias_ap)
            r0 += nr

    def pool2(src3, dst3, orows, ocols, cch):
        t1 = tmp.tile([cch, orows * ocols], F32R, tag="pool_a", name="poolt1")
        t2 = tmp.tile([cch, orows * ocols], F32R, tag="pool_b", name="poolt2")
        v = [src3[:, 1 + a:1 + a + 2 * orows:2, 1 + b:1 + b + 2 * ocols:2]
             for a, b in ((0, 0), (1, 1), (0, 1), (1, 0))]
        nc.vector.tensor_max(_r3(t1[:], orows, ocols), v[0], v[1])
        nc.vector.tensor_max(_r3(t2[:], orows, ocols), v[2], v[3])
        nc.vector.tensor_max(dst3[:, 1:1 + orows, 1:1 + ocols],
                             _r3(t1[:], orows, ocols), _r3(t2[:], orows, ocols))

    def up2(src3, dst3, irows, icols):
        s = src3[:, 1:1 + irows, 1:1 + icols]
        for a in (0, 1):
            for b in (0, 1):
                nc.vector.tensor_copy(
                    dst3[:, 1 + a:1 + a + 2 * irows:2,
                         1 + b:1 + b + 2 * icols:2], s)

    def embconv(plane3, rows, imbuf, dst, scale):
        """1->100 3x3 conv via K=10 im2col matmul (row 9 = ones, wt row 9 =
        emb bias). plane3=None -> imbuf is prebuilt."""
        n = rows * W
        if plane3 is not None:
            for s in range(9):
                dy, dx = s // 3, s % 3
                dma(imbuf[s:s + 1, 0:n], plane3[0:1, dy:dy + rows, dx:dx + W])
        nch = 6 if rows == H else 3
        cw = n // nch
        for ci in range(nch):
            ps = pconv.tile([100, cw], F32, tag="conv", name="convps")
            nc.tensor.matmul(ps[:], wt["emb"],
                             imbuf[:, ci * cw:(ci + 1) * cw],
                             start=True, stop=True)
            nc.scalar.activation(dst[0:100, ci * cw:(ci + 1) * cw], ps[:],
                                 AF.Copy, scale=scale)

    def sqrow(ex, rowbuf):
        nc.scalar.activation(esq[:, 0:NPIX], ex[0:100, 0:NPIX], AF.Square)
        cw = NPIX // 6
        for ci in range(6):
            ps = pconv.tile([1, cw], F32, tag="conv", name="sqps")
            nc.tensor.matmul(ps[:], c025[:],
                             esq[:, ci * cw:(ci + 1) * cw],
                             start=True, stop=True)
            nc.scalar.copy(rowbuf[0:1, ci * cw:(ci + 1) * cw], ps[:])
        dma(ex[100:101, 0:NPIX], rowbuf[0:1, 0:NPIX])

    # ------------------------------------------------------------- U-Net
    e1p3 = _r3(e1p[:], 50, 50)
    p1p3 = _r3(p1p[:], 26, 26)
    e2p3 = _r3(e2p[:], 26, 26)
    p2p3 = _r3(p2p[:], 14, 14)
    btp3 = _r3(btp[:], 14, 14)
    ubp3 = _r3(ubp[:], 26, 26)
    d2p3 = _r3(d2p[:], 26, 26)
    udp3 = _r3(udp[:], 50, 50)
    d1p3 = _r3(d1p[:], 50, 50)

    conv_im2col([(im27[:], wt["enc1s"])], 16, [10, 10, 10, 8], W,
                AF.Relu, bia["enc1"], e1p3)
    pool2(e1p3, p1p3, 19, 24, 16)
    conv9([(p1p3, wt["enc2"], 16)], 32, [18], 24, AF.Relu, bia["enc2"], e2p3)
    pool2(e2p3, p2p3, 9, 12, 32)
    conv9([(p2p3, wt["bott"], 32)], 64, [8], 12, AF.Relu, bia["bott"], btp3)
    up2(btp3, ubp3, 8, 12)
    conv9([(ubp3, wt["dec2a"], 64), (e2p3, wt["dec2b"], 32)], 32, [14], 24,
          AF.Relu, bia["dec2"], d2p3)
    up2(d2p3, udp3, 14, 24)
    conv9([(udp3, wt["dec1a"], 32), (e1p3, wt["dec1b"], 16)], 16, [10, 10, 6],
          W, AF.Relu, bia["dec1"], d1p3)

    # 1x1 output conv -> xt[0:4] (all 4 channels) and x3cp (class-c channel)
    r0 = 0
    for nr in (10, 10, 6):
        rhs = d1p3[:, 1 + r0:1 + r0 + nr, 1:1 + W]
        ps = pconv.tile([4, nr * W], F32, tag="conv", name="convps")
        nc.tensor.matmul(ps[:], wt["out"], rhs.bitcast(F32R),
                         start=True, stop=True)
        nc.scalar.activation(xt3[0:4, 1 + r0:1 + r0 + nr, 1:1 + W],
                             _r3(ps[:], nr, W), AF.Identity, bias=bia["out"])
        psc = pconv.tile([1, nr * W], F32, tag="conv", name="convps")
        nc.tensor.matmul(psc[:], wt["outc"], rhs.bitcast(F32R),
                         start=True, stop=True)
        nc.scalar.activation(x3cp3[0:1, 1 + r0:1 + r0 + nr, 1:1 + W],
                             _r3(psc[:], nr, W), AF.Identity,
                             bias=bia["outc"])
        r0 += nr

    # early head-conv im2col (channels 4/5 hold stale zeros; patched later)
    im2col_build(imdsh[:], xt3, 24, W, 7)

    # ------------------------------------------------- embeddings (filler)
    embconv(None, H, im2c12[0:10, 0:NPIX], e1x[:], -2.0)
    sqrow(e1x[:], a2row1[:])
    embconv(None, H, im2c12[0:10, NPIX:2 * NPIX], e2x[:], -2.0)
    sqrow(e2x[:], a2row2[:])

    # ------------------------------------------------------- embedding 3
    embconv(x3cp3, QROWS, im2c3[:], e3x[:], 1.0)
    nc.scalar.activation(esq[:, 0:Q], e3x[0:100, 0:Q], AF.Square)

    # ------------------------------------------------------- matching
    # P chunks: two [120,1024] psum tiles (2 matmuls each) + one [120,256]
    for c in range(NQC):
        lhsT = e3x[:, c * QCH:(c + 1) * QCH]
        b2ps = pconv.tile([QCH, 2], F32, tag="conv", name="b2ps")
        nc.tensor.matmul(b2ps[:], esq[:, c * QCH:(c + 1) * QCH], c1[:],
                         start=True, stop=True)
        b2c = small.tile([QCH, 1], F32, tag="b2c", name="b2c")
        nc.scalar.copy(b2c[:], b2ps[:, 0:1])
        for r, ex in enumerate((e1x, e2x)):
            mins = small.tile([QCH, 3], F32, tag="mins", name="mins")
            for j in range(2):
                ps = pmain.tile([QCH, 1024], F32, tag="main", name="mainps")
                nc.tensor.matmul(ps[:, 0:512], lhsT,
                                 ex[:][:, j * 1024:j * 1024 + 512],
                                 start=True, stop=True)
                nc.tensor.matmul(ps[:, 512:1024], lhsT,
                                 ex[:][:, j * 1024 + 512:(j + 1) * 1024],
                                 start=True, stop=True)
                nc.vector.tensor_reduce(mins[:, j:j + 1], ps[:],
                                        axis=AX.X, op=ALU.min)
            pt = pmain.tile([QCH, 256], F32, tag="maint", name="maintps", bufs=2)
            nc.tensor.matmul(pt[:], lhsT, ex[:][:, 2048:2304],
                             start=True, stop=True)
            nc.vector.tensor_reduce(mins[:, 2:3], pt[:], axis=AX.X,
                                    op=ALU.min)
            dmin = small.tile([QCH, 1], F32, tag="dmin", name="dmin")
            nc.vector.tensor_reduce(dmin[:], mins[:], axis=AX.X, op=ALU.min)
            dmax = small.tile([QCH, 1], F32, tag="dmax", name="dmax")
            nc.vector.tensor_scalar(dmax[:], dmin[:], b2c[:], 0.0,
                                    op0=ALU.add, op1=ALU.max)
            gcol = small.tile([QCH, 1], F32, tag="gcol", name="gcol")
            nc.scalar.activation(gcol[:], dmax[:], AF.Tanh, scale=0.5)
            # transpose this chunk's column into the flat row now
            pst = pconv.tile([1, QCH], F32, tag="conv", name="gmtps")
            nc.tensor.transpose(pst[:], gcol[:], ident[:QCH, :QCH])
            flat = gflat if r == 0 else lflat
            nc.scalar.copy(flat[0:1, c * QCH:(c + 1) * QCH], pst[:])

    # gm/lm rows -> xt planes 4/5, patch imdsh channels 4/5, head conv
    dma(xt3[4:5, 1:26, 1:49], _r3(gflat[:], QROWS, W)[:, :, :])
    im2col_build(imdsh[:], xt3, 24, W, 7, chans=(4, 5))
    dma(xt3[5:6, 1:26, 1:49], _r3(lflat[:], QROWS, W)[:, :, :])
    im2col_build(imdsh[:], xt3, 24, W, 7, chans=(5, 6))
    r0 = 0
    for nr in (8, 8, 8):
        ps = pconv.tile([1, nr * W], F32, tag="conv", name="convps")
        nc.tensor.matmul(ps[:], wt["dsh"],
                         imdsh[:, r0 * W:(r0 + nr) * W],
                         start=True, stop=True)
        nc.scalar.activation(out_sb[0:1, r0 * W:(r0 + nr) * W],
                             _r3(ps[:], nr, W), AF.Identity,
                             bias=bia["dsh"])
        r0 += nr
    nc.sync.dma_start(out_d, out_sb[:])


def build_program():
    import contextlib
    nc = bacc.Bacc("TRN2", target_bir_lowering=False, debug=False,
                   num_devices=8)
    with tile.TileContext(nc) as tc:
        with contextlib.ExitStack() as ctx:
            _emit(nc, tc, ctx)
    nc.compile()
    return nc


def _get_program():
    global _PROG
    if _PROG is None:
        _PROG = build_program()
    return _PROG


CORE_BC = [(0, 2), (0, 3), (1, 2), (1, 3)]


def _wT_flat(w):
    """[Cout, Cin, 3, 3] -> [Cin, 9*Cout]: col block s holds w[:, :, s//3, s%3].T"""
    cout, cin = w.shape[:2]
    out = np.zeros((cin, 9 * cout), np.float32)
    for s in range(9):
        out[:, s * cout:(s + 1) * cout] = w[:, :, s // 3, s % 3].T
    return out


def _pad50(img):
    out = np.zeros((50, 50), np.float32)
    out[1:49, 1:49] = img
    return out


def _im2col9(img, rows, ones_row=False):
    """padded 50x50 -> [9(+1), rows*48] rows ordered s=dy*3+dx."""
    p = _pad50(img)
    rws = [p[dy:dy + rows, dx:dx + W].ravel()
           for dy in range(3) for dx in range(3)]
    if ones_row:
        rws.append(np.ones(rows * W, np.float32))
    return np.stack(rws)


def _blobw(inp, flip, c):
    w = {k: (inp[k][:, :, ::-1, :] if flip else inp[k])
         for k in ["enc1_w", "enc2_w", "bott_w", "dec2_w", "dec1_w",
                   "emb_w", "dsh_w"]}
    seg = {}
    seg["enc1s"] = w["enc1_w"].reshape(16, 3, 9).transpose(2, 1, 0) \
                              .reshape(27, 16)
    seg["enc2"] = _wT_flat(w["enc2_w"])
    seg["bott"] = _wT_flat(w["bott_w"])
    seg["dec2a"] = _wT_flat(w["dec2_w"][:, :64])
    seg["dec2b"] = _wT_flat(w["dec2_w"][:, 64:])
    seg["dec1a"] = _wT_flat(w["dec1_w"][:, :32])
    seg["dec1b"] = _wT_flat(w["dec1_w"][:, 32:])
    seg["out"] = inp["out_w"][:, :, 0, 0].T
    seg["emb"] = np.vstack([w["emb_w"].reshape(100, 9).T,
                            inp["emb_b"][None, :]])
    seg["dsh"] = w["dsh_w"].reshape(7, 9).T.reshape(63, 1)
    seg["outc"] = inp["out_w"][c, :, 0, 0][:, None]
    for k in ["enc1", "enc2", "bott", "dec2", "dec1", "out", "dsh"]:
        seg["b_" + k] = inp[k + "_b"][:, None]
    seg["b_outc"] = inp["out_b"][c:c + 1][:, None]
    blob = np.zeros((64, WCOLS), np.float32)
    for nm, rows, cols in WSEGS:
        blob[0:rows, WOFF[nm]:WOFF[nm] + cols] = seg[nm]
    return blob


def make_in_maps(inp):
    maps = []
    for k8 in range(8):
        n_idx, half = k8 // 2, k8 % 2
        b, c = CORE_BC[n_idx]
        x1c, x2c, x3b = inp["x1"][b, c], inp["x2"][b, c], inp["x3"][b]
        if half:
            x1c, x2c, x3b = x1c[::-1], x2c[::-1], x3b[:, ::-1]
        # enc1 im2col [27, 39*48]: row s*3+ci
        bx1 = np.zeros((27, 39 * W), np.float32)
        for ci in range(3):
            im9 = _im2col9(x3b[ci], 39)
            for s in range(9):
                bx1[s * 3 + ci] = im9[s]
        bx2 = np.concatenate([_im2col9(x1c, H, True),
                              _im2col9(x2c, H, True)], axis=1)
        bx3 = x2c[0:25, :].reshape(1, Q)
        maps.append({"blobW": np.ascontiguousarray(_blobw(inp, bool(half), c)),
                     "blobX1": np.ascontiguousarray(bx1),
                     "blobX2": np.ascontiguousarray(bx2),
                     "blobX3": np.ascontiguousarray(bx3)})
    return maps


def assemble(results):
    out = np.zeros((2, 2, H, W), np.float32)
    for k8, r in enumerate(results):
        n_idx, half = k8 // 2, k8 % 2
        b, c = CORE_BC[n_idx]
        y = r["out"]
        if half == 0:
            out[b, c - 2, 0:24] = y
        else:
            out[b, c - 2, 24:48] = y[::-1]
    return out


def kernel(**inputs):
    inp = {k: np.asarray(v) for k, v in inputs.items()}
    nc = _get_program()
    maps = make_in_maps(inp)
    res = run_bass_kernel_spmd(nc, maps, core_ids=list(range(8)), trace=False)
    return assemble(res.results)


# revision 30
# speedup vs baseline: 1.1562x; 1.0254x over previous
"""FEELVOS fused kernel for TRN2, 8-core SPMD.

Sharding: the reference only returns logits for classes C-2, C-1, so only 4 of
the 8 fused (batch, class) items matter. 8 cores = 4 (b, c) pairs x 2 frame
halves (top/bottom 24 rows). Bottom-half cores receive row-flipped inputs and
row-flipped conv kernels so every core runs the identical program computing
"top 25 rows" of its (possibly flipped) frame; the host un-flips on gather.

Per core:
  - partial U-Net on x3[b] (top-aligned row windows; convs as im2col or
    9-shifted matmuls on zero-padded [C, 50x50] SBUF planes, fp32r)
  - 100-d embeddings of x1[b,c], x2[b,c] (full frame, ref side) and of the
    class-c channel of the U-Net output (25-row window, query side)
  - distance matrix via one K=101 matmul per tile: rows 0..99 carry
    e3 . (-2 e_ref), row 100 carries 1 * |a_p|^2; DVE reduce_min over the
    free axis; |b_q|^2 added after the min (per-chunk column matmul);
    then tanh(d/2) == 1 - 2*sigmoid(-d)
  - 3x3 head conv (im2col, K=63) over [x3s(4ch), gm, lm, x2] -> 24 rows.
"""
import numpy as np

import concourse.bass as bass
import concourse.bacc as bacc
import concourse.tile as tile
from concourse import mybir
from concourse.bass_utils import run_bass_kernel_spmd
from concourse.masks import make_identity
from concourse.tile import add_dep_helper

F32 = mybir.dt.float32
F32R = mybir.dt.float32r
AF = mybir.ActivationFunctionType
ALU = mybir.AluOpType
AX = mybir.AxisListType

H = W = 48
NPIX = H * W                 # 2304 ref pixels
QROWS = 25
Q = QROWS * W                # 1200 query pixels
QCH, NQC = 120, 10           # query chunking for the distance matmul
PCH = [(0, 512), (512, 512), (1024, 512), (1536, 512), (2048, 256)]

_PROG = None


def _r3(ap, h, w):
    return ap.rearrange("c (h w) -> c h w", h=h, w=w)



# blobW column layout: (name, rows, cols); all segments base partition 0.
WSEGS = [("enc1s", 27, 16), ("enc2", 16, 288), ("bott", 32, 576),
         ("dec2a", 64, 288), ("dec2b", 32, 288), ("dec1a", 32, 144),
         ("dec1b", 16, 144), ("out", 16, 4), ("emb", 10, 100),
         ("dsh", 63, 1), ("outc", 16, 1),
         ("b_enc1", 16, 1), ("b_enc2", 32, 1), ("b_bott", 64, 1),
         ("b_dec2", 32, 1), ("b_dec1", 16, 1), ("b_out", 4, 1),
         ("b_dsh", 1, 1), ("b_outc", 1, 1)]
WOFF = {}
_o = 0
for _nm, _r, _c in WSEGS:
    WOFF[_nm] = _o
    _o += _c
WCOLS = _o


def _emit(nc, tc, ctx):
    # ------------------------------------------------------------- dram io
    bW = nc.dram_tensor("blobW", [64, WCOLS], F32, kind="ExternalInput").ap()
    bX1 = nc.dram_tensor("blobX1", [27, 39 * W], F32,
                         kind="ExternalInput").ap()
    bX2 = nc.dram_tensor("blobX2", [10, 2 * NPIX], F32,
                         kind="ExternalInput").ap()
    bX3 = nc.dram_tensor("blobX3", [1, Q], F32, kind="ExternalInput").ap()
    out_d = nc.dram_tensor("out", [24, W], F32, kind="ExternalOutput").ap()

    # round-robin DMA dispatch over the two HWDGE engines
    _dmaq = [nc.sync, nc.scalar]
    _qi = [0]

    def dma(out, in_):
        eng = _dmaq[_qi[0] % len(_dmaq)]
        _qi[0] += 1
        return eng.dma_start(out, in_)

    # ------------------------------------------------------------- sbuf
    sb = ctx.enter_context(tc.tile_pool(name="sb", bufs=1))

    def st(name, p, f, dt=F32):
        return sb.tile([p, f], dt, tag=name, name=name)

    blobw = st("blobw", 64, WCOLS, F32R)
    im27 = st("im27", 27, 39 * W, F32R)     # enc1 im2col (host-built)
    im2c12 = st("im2c12", 10, 2 * NPIX, F32R)  # emb im2col e1|e2 (host-built)
    xt6row = st("xt6row", 1, Q, F32R)

    def wseg(nm):
        rows, cols = next((r, c) for n, r, c in WSEGS if n == nm)
        return blobw[0:rows, WOFF[nm]:WOFF[nm] + cols]

    wt = {nm: wseg(nm) for nm, _, _ in WSEGS if not nm.startswith("b_")}
    bia = {nm[2:]: wseg(nm).bitcast(F32)
           for nm, _, _ in WSEGS if nm.startswith("b_")}

    # device-written padded planes
    x3cp = st("x3cp", 1, 2500, F32R)
    e1p = st("e1p", 16, 2500, F32R)
    p1p = st("p1p", 16, 676, F32R)
    e2p = st("e2p", 32, 676, F32R)
    p2p = st("p2p", 32, 196, F32R)
    btp = st("btp", 64, 196, F32R)
    ubp = st("ubp", 64, 676, F32R)
    d2p = st("d2p", 32, 676, F32R)
    udp = st("udp", 32, 2500, F32R)
    d1p = st("d1p", 16, 2500, F32R)
    xt = st("xt", 7, 2500, F32R)

    im2c3 = st("im2c3", 10, Q, F32R)        # emb im2col (e3)
    imdsh = st("imdsh", 63, 24 * W, F32R)   # head im2col (rows s*7+ci)
    e1x = st("e1x", 101, NPIX, F32R)
    e2x = st("e2x", 101, NPIX, F32R)
    e3x = st("e3x", 101, Q, F32R)
    esq = st("esq", 100, NPIX, F32R)
    ident = st("ident", 128, 128)

    c025 = st("c025", 100, 1, F32R)
    c1 = st("c1", 100, 2, F32R)
    a2row1 = st("a2row1", 1, NPIX, F32R)
    a2row2 = st("a2row2", 1, NPIX, F32R)
    gflat = st("gflat", 1, Q, F32R)
    lflat = st("lflat", 1, Q, F32R)
    out_sb = st("out_sb", 1, 24 * W)

    small = ctx.enter_context(tc.tile_pool(name="small", bufs=8))
    tmp = ctx.enter_context(tc.tile_pool(name="tmp", bufs=2))

    # ------------------------------------------------------------- init
    dma(im27[:], bX1.bitcast(F32R))
    dma(blobw[:], bW.bitcast(F32R))
    dma(im2c12[:], bX2.bitcast(F32R))
    dma(xt6row[:], bX3.bitcast(F32R))
    make_identity(nc, ident[:])

    xt3 = _r3(xt[:], 50, 50)
    x3cp3 = _r3(x3cp[:], 50, 50)

    # borders of device-written planes; xt fully (ch 4/5 are read by the
    # early imdsh build before gm/lm land, then patched)
    def borders(t, pw):
        v = _r3(t[:], pw, pw).bitcast(F32)
        return [nc.gpsimd.memset(v[:, 0:1, :], 0.0),
                nc.gpsimd.memset(v[:, pw - 1:pw, :], 0.0),
                nc.gpsimd.memset(v[:, 1:pw - 1, 0:1], 0.0),
                nc.gpsimd.memset(v[:, 1:pw - 1, pw - 1:pw], 0.0)]

    h_xt_ms = nc.gpsimd.memset(xt[:].bitcast(F32), 0.0)
    h_x3cp_b = borders(x3cp, 50)
    for t, pw in ((e1p, 50), (p1p, 26), (e2p, 26), (p2p, 14),
                  (btp, 14), (ubp, 26), (d2p, 26), (udp, 50)):
        borders(t, pw)
    nc.vector.memset(e3x[96:101, :].bitcast(F32), 1.0)   # row 100 = ones
    nc.vector.memset(im2c3[:].bitcast(F32), 1.0)
    nc.gpsimd.memset(c025[:].bitcast(F32), 0.25)
    nc.gpsimd.memset(c1[:].bitcast(F32), 1.0)
    # xt channel 6 = x2 rows 0..24
    h_xt6 = dma(xt3[6:7, 1:26, 1:49], xt6row[:])

    pconv = ctx.enter_context(tc.tile_pool(name="pconv", bufs=2, space="PSUM"))
    pmain = ctx.enter_context(tc.tile_pool(name="pmain", bufs=2, space="PSUM"))

    # ------------------------------------------------------------ helpers
    def conv9(srcs, cout, row_chunks, w_, func, bias_ap, dst3, scale=1.0):
        """3x3 conv via 9 shifted matmuls accumulating in PSUM.
        srcs: list of (plane3d, wtile, cin)."""
        r0 = 0
        for nr in row_chunks:
            ps = pconv.tile([cout, nr * w_], F32, tag="conv", name="convps")
            ops = []
            for (src3, wtile, cin) in srcs:
                for s in range(9):
                    dy, dx = s // 3, s % 3
                    ops.append((wtile[0:cin, s * cout:(s + 1) * cout],
                                src3[:, r0 + dy:r0 + dy + nr, dx:dx + w_]))
            for i, (l, r) in enumerate(ops):
                nc.tensor.matmul(ps[:], l, r.bitcast(F32R),
                                 start=(i == 0), stop=(i == len(ops) - 1))
            nc.scalar.activation(dst3[:, 1 + r0:1 + r0 + nr, 1:1 + w_],
                                 _r3(ps[:], nr, w_), func,
                                 bias=bias_ap, scale=scale)
            r0 += nr

    def shift_build(dst, plane3, ci, row0, rstep, nrows):
        """9 shift-DMAs: dst[row0 + s*rstep] = channel-ci window (dy, dx)."""
        for s in range(9):
            dy, dx = s // 3, s % 3
            dma(dst[row0 + s * rstep:row0 + s * rstep + 1, 0:nrows * W],
                plane3[ci:ci + 1, dy:dy + nrows, dx:dx + W])

    def conv_im2col(imbufs, cout, row_chunks, w_, func, bias_ap, dst3):
        r0 = 0
        for nr in row_chunks:
            ps = pconv.tile([cout, nr * w_], F32, tag="conv", name="convps")
            for i, (im, lhsT) in enumerate(imbufs):
                nc.tensor.matmul(ps[:], lhsT, im[:, r0 * w_:(r0 + nr) * w_],
                                 start=(i == 0), stop=(i == len(imbufs) - 1))
            nc.scalar.activation(dst3[:, 1 + r0:1 + r0 + nr, 1:1 + w_],
                                 _r3(ps[:], nr, w_), func, bias=bias_ap)
            r0 += nr

    def pool2(src3, dst3, orows, ocols, cch):
        t1 = tmp.tile([cch, orows * ocols], F32R, tag="pool_a", name="poolt1")
        t2 = tmp.tile([cch, orows * ocols], F32R, tag="pool_b", name="poolt2")
        v = [src3[:, 1 + a:1 + a + 2 * orows:2, 1 + b:1 + b + 2 * ocols:2]
             for a, b in ((0, 0), (1, 1), (0, 1), (1, 0))]
        nc.vector.tensor_max(_r3(t1[:], orows, ocols), v[0], v[1])
        nc.vector.tensor_max(_r3(t2[:], orows, ocols), v[2], v[3])
        nc.vector.tensor_max(dst3[:, 1:1 + orows, 1:1 + ocols],
                             _r3(t1[:], orows, ocols), _r3(t2[:], orows, ocols))

    def up2(src3, dst3, irows, icols):
        s = src3[:, 1:1 + irows, 1:1 + icols]
        for a in (0, 1):
            for b in (0, 1):
                nc.vector.tensor_copy(
                    dst3[:, 1 + a:1 + a + 2 * irows:2,
                         1 + b:1 + b + 2 * icols:2], s)

    def embconv(plane3, rows, imbuf, dst, scale):
        """1->100 3x3 conv via K=10 im2col matmul (row 9 = ones, wt row 9 =
        emb bias). plane3=None -> imbuf is prebuilt."""
        n = rows * W
        if plane3 is not None:
            for s in range(9):
                dy, dx = s // 3, s % 3
                dma(imbuf[s:s + 1, 0:n], plane3[0:1, dy:dy + rows, dx:dx + W])
        nch = 6 if rows == H else 3
        cw = n // nch
        for ci in range(nch):
            ps = pconv.tile([100, cw], F32, tag="conv", name="convps")
            nc.tensor.matmul(ps[:], wt["emb"],
                             imbuf[:, ci * cw:(ci + 1) * cw],
                             start=True, stop=True)
            nc.scalar.activation(dst[0:100, ci * cw:(ci + 1) * cw], ps[:],
                                 AF.Copy, scale=scale)

    def sqrow(ex, rowbuf):
        nc.scalar.activation(esq[:, 0:NPIX], ex[0:100, 0:NPIX], AF.Square)
        cw = NPIX // 6
        for ci in range(6):
            ps = pconv.tile([1, cw], F32, tag="conv", name="sqps")
            nc.tensor.matmul(ps[:], c025[:],
                             esq[:, ci * cw:(ci + 1) * cw],
                             start=True, stop=True)
            nc.scalar.copy(rowbuf[0:1, ci * cw:(ci + 1) * cw], ps[:])
        dma(ex[100:101, 0:NPIX], rowbuf[0:1, 0:NPIX])

    # ------------------------------------------------------------- U-Net
    e1p3 = _r3(e1p[:], 50, 50)
    p1p3 = _r3(p1p[:], 26, 26)
    e2p3 = _r3(e2p[:], 26, 26)
    p2p3 = _r3(p2p[:], 14, 14)
    btp3 = _r3(btp[:], 14, 14)
    ubp3 = _r3(ubp[:], 26, 26)
    d2p3 = _r3(d2p[:], 26, 26)
    udp3 = _r3(udp[:], 50, 50)
    d1p3 = _r3(d1p[:], 50, 50)

    conv_im2col([(im27[:], wt["enc1s"])], 16, [10, 10, 10, 8], W,
                AF.Relu, bia["enc1"], e1p3)
    pool2(e1p3, p1p3, 19, 24, 16)
    conv9([(p1p3, wt["enc2"], 16)], 32, [18], 24, AF.Relu, bia["enc2"], e2p3)
    pool2(e2p3, p2p3, 9, 12, 32)
    conv9([(p2p3, wt["bott"], 32)], 64, [8], 12, AF.Relu, bia["bott"], btp3)
    up2(btp3, ubp3, 8, 12)
    conv9([(ubp3, wt["dec2a"], 64), (e2p3, wt["dec2b"], 32)], 32, [14], 24,
          AF.Relu, bia["dec2"], d2p3)
    up2(d2p3, udp3, 14, 24)
    conv9([(udp3, wt["dec1a"], 32), (e1p3, wt["dec1b"], 16)], 16, [10, 10, 6],
          W, AF.Relu, bia["dec1"], d1p3)

    # 1x1 output conv -> xt[0:4] (all 4 channels) and x3cp (class-c channel)
    r0 = 0
    h_xt_ep, h_x3cp_ep = [], []
    for nr in (10, 10, 6):
        rhs = d1p3[:, 1 + r0:1 + r0 + nr, 1:1 + W]
        ps = pconv.tile([4, nr * W], F32, tag="conv", name="convps")
        nc.tensor.matmul(ps[:], wt["out"], rhs.bitcast(F32R),
                         start=True, stop=True)
        h_xt_ep.append(nc.scalar.activation(
            xt3[0:4, 1 + r0:1 + r0 + nr, 1:1 + W],
            _r3(ps[:], nr, W), AF.Identity, bias=bia["out"]))
        psc = pconv.tile([1, nr * W], F32, tag="conv", name="convps")
        nc.tensor.matmul(psc[:], wt["outc"], rhs.bitcast(F32R),
                         start=True, stop=True)
        h_x3cp_ep.append(nc.scalar.activation(
            x3cp3[0:1, 1 + r0:1 + r0 + nr, 1:1 + W],
            _r3(psc[:], nr, W), AF.Identity, bias=bia["outc"]))
        r0 += nr

    # early head-conv im2col, rows s*7+ci (gm/lm rows hold zeros from the
    # xt memset; patched after the matching).
    for s in range(9):
        dy, dx = s // 3, s % 3
        dma(imdsh[s * 7:(s + 1) * 7, 0:24 * W],
            xt3[:, dy:dy + 24, dx:dx + W])

    # ------------------------------------------------- embeddings (filler)
    embconv(None, H, im2c12[0:10, 0:NPIX], e1x[:], -2.0)
    sqrow(e1x[:], a2row1[:])
    embconv(None, H, im2c12[0:10, NPIX:2 * NPIX], e2x[:], -2.0)
    sqrow(e2x[:], a2row2[:])

    # ------------------------------------------------------- embedding 3
    shift_build(im2c3[:], x3cp3, 0, 0, 1, QROWS)
    embconv(None, QROWS, im2c3[:], e3x[:], 1.0)
    nc.scalar.activation(esq[:, 0:Q], e3x[0:100, 0:Q], AF.Square)

    # ------------------------------------------------------- matching
    # ref-major: all gm chunks first so the gm plane + its imdsh patch DMAs
    # hide behind lm's compute. |b|^2 columns computed once, cached.
    b2cs = []
    for c in range(NQC):
        b2ps = pconv.tile([QCH, 2], F32, tag="conv", name="b2ps")
        nc.tensor.matmul(b2ps[:], esq[:, c * QCH:(c + 1) * QCH], c1[:],
                         start=True, stop=True)
        b2c = small.tile([QCH, 1], F32, tag="b2c", name="b2c", bufs=10)
        nc.scalar.copy(b2c[:], b2ps[:, 0:1])
        b2cs.append(b2c)
    for r, ex in enumerate((e1x, e2x)):
        flat = gflat if r == 0 else lflat
        for c in range(NQC):
            lhsT = e3x[:, c * QCH:(c + 1) * QCH]
            mins = small.tile([QCH, 3], F32, tag="mins", name="mins")
            for j in range(2):
                ps = pmain.tile([QCH, 1024], F32, tag="main", name="mainps")
                nc.tensor.matmul(ps[:, 0:512], lhsT,
                                 ex[:][:, j * 1024:j * 1024 + 512],
                                 start=True, stop=True)
                nc.tensor.matmul(ps[:, 512:1024], lhsT,
                                 ex[:][:, j * 1024 + 512:(j + 1) * 1024],
                                 start=True, stop=True)
                nc.vector.tensor_reduce(mins[:, j:j + 1], ps[:],
                                        axis=AX.X, op=ALU.min)
            pt = pmain.tile([QCH, 256], F32, tag="maint", name="maintps",
                            bufs=2)
            nc.tensor.matmul(pt[:], lhsT, ex[:][:, 2048:2304],
                             start=True, stop=True)
            nc.vector.tensor_reduce(mins[:, 2:3], pt[:], axis=AX.X,
                                    op=ALU.min)
            dmin = small.tile([QCH, 1], F32, tag="dmin", name="dmin")
            nc.vector.tensor_reduce(dmin[:], mins[:], axis=AX.X, op=ALU.min)
            dmax = small.tile([QCH, 1], F32, tag="dmax", name="dmax")
            nc.vector.tensor_scalar(dmax[:], dmin[:], b2cs[c][:], 0.0,
                                    op0=ALU.add, op1=ALU.max)
            gcol = small.tile([QCH, 1], F32, tag="gcol", name="gcol")
            nc.scalar.activation(gcol[:], dmax[:], AF.Tanh, scale=0.5)
            pst = pconv.tile([1, QCH], F32, tag="conv", name="gmtps")
            nc.tensor.transpose(pst[:], gcol[:], ident[:QCH, :QCH])
            nc.scalar.copy(flat[0:1, c * QCH:(c + 1) * QCH], pst[:])
        # plane write + imdsh patch for this ref
        plane_h = dma(xt3[4 + r:5 + r, 1:26, 1:49],
                      _r3(flat[:], QROWS, W)[:, :, :])
        shift_build(imdsh[:], xt3, 4 + r, 4 + r, 7, 24)

    r0 = 0
    for nr in (8, 8, 8):
        ps = pconv.tile([1, nr * W], F32, tag="conv", name="convps")
        nc.tensor.matmul(ps[:], wt["dsh"],
                         imdsh[:, r0 * W:(r0 + nr) * W],
                         start=True, stop=True)
        nc.scalar.activation(out_sb[0:1, r0 * W:(r0 + nr) * W],
                             _r3(ps[:], nr, W), AF.Identity,
                             bias=bia["dsh"])
        r0 += nr
    nc.sync.dma_start(out_d, out_sb[:])


def build_program():
    import contextlib
    nc = bacc.Bacc("TRN2", target_bir_lowering=False, debug=False,
                   num_devices=8)
    with tile.TileContext(nc) as tc:
        with contextlib.ExitStack() as ctx:
            _emit(nc, tc, ctx)
    nc.compile()
    return nc


def _get_program():
    global _PROG
    if _PROG is None:
        _PROG = build_program()
    return _PROG


CORE_BC = [(0, 2), (0, 3), (1, 2), (1, 3)]


def _wT_flat(w):
    """[Cout, Cin, 3, 3] -> [Cin, 9*Cout]: col block s holds w[:, :, s//3, s%3].T"""
    cout, cin = w.shape[:2]
    out = np.zeros((cin, 9 * cout), np.float32)
    for s in range(9):
        out[:, s * cout:(s + 1) * cout] = w[:, :, s // 3, s % 3].T
    return out


def _pad50(img):
    out = np.zeros((50, 50), np.float32)
    out[1:49, 1:49] = img
    return out


def _im2col9(img, rows, ones_row=False):
    """padded 50x50 -> [9(+1), rows*48] rows ordered s=dy*3+dx."""
    p = _pad50(img)
    rws = [p[dy:dy + rows, dx:dx + W].ravel()
           for dy in range(3) for dx in range(3)]
    if ones_row:
        rws.append(np.ones(rows * W, np.float32))
    return np.stack(rws)


def _blobw(inp, flip, c):
    w = {k: (inp[k][:, :, ::-1, :] if flip else inp[k])
         for k in ["enc1_w", "enc2_w", "bott_w", "dec2_w", "dec1_w",
                   "emb_w", "dsh_w"]}
    seg = {}
    seg["enc1s"] = w["enc1_w"].reshape(16, 3, 9).transpose(2, 1, 0) \
                              .reshape(27, 16)
    seg["enc2"] = _wT_flat(w["enc2_w"])
    seg["bott"] = _wT_flat(w["bott_w"])
    seg["dec2a"] = _wT_flat(w["dec2_w"][:, :64])
    seg["dec2b"] = _wT_flat(w["dec2_w"][:, 64:])
    seg["dec1a"] = _wT_flat(w["dec1_w"][:, :32])
    seg["dec1b"] = _wT_flat(w["dec1_w"][:, 32:])
    seg["out"] = inp["out_w"][:, :, 0, 0].T
    seg["emb"] = np.vstack([w["emb_w"].reshape(100, 9).T,
                            inp["emb_b"][None, :]])
    seg["dsh"] = w["dsh_w"].reshape(7, 9).T.reshape(63, 1)
    seg["outc"] = inp["out_w"][c, :, 0, 0][:, None]
    for k in ["enc1", "enc2", "bott", "dec2", "dec1", "out", "dsh"]:
        seg["b_" + k] = inp[k + "_b"][:, None]
    seg["b_outc"] = inp["out_b"][c:c + 1][:, None]
    blob = np.zeros((64, WCOLS), np.float32)
    for nm, rows, cols in WSEGS:
        blob[0:rows, WOFF[nm]:WOFF[nm] + cols] = seg[nm]
    return blob


def make_in_maps(inp):
    maps = []
    for k8 in range(8):
        n_idx, half = k8 // 2, k8 % 2
        b, c = CORE_BC[n_idx]
        x1c, x2c, x3b = inp["x1"][b, c], inp["x2"][b, c], inp["x3"][b]
        if half:
            x1c, x2c, x3b = x1c[::-1], x2c[::-1], x3b[:, ::-1]
        # enc1 im2col [27, 39*48]: row s*3+ci
        bx1 = np.zeros((27, 39 * W), np.float32)
        for ci in range(3):
            im9 = _im2col9(x3b[ci], 39)
            for s in range(9):
                bx1[s * 3 + ci] = im9[s]
        bx2 = np.concatenate([_im2col9(x1c, H, True),
                              _im2col9(x2c, H, True)], axis=1)
        bx3 = x2c[0:25, :].reshape(1, Q)
        maps.append({"blobW": np.ascontiguousarray(_blobw(inp, bool(half), c)),
                     "blobX1": np.ascontiguousarray(bx1),
                     "blobX2": np.ascontiguousarray(bx2),
                     "blobX3": np.ascontiguousarray(bx3)})
    return maps


def assemble(results):
    out = np.zeros((2, 2, H, W), np.float32)
    for k8, r in enumerate(results):
        n_idx, half = k8 // 2, k8 % 2
        b, c = CORE_BC[n_idx]
        y = r["out"]
        if half == 0:
            out[b, c - 2, 0:24] = y
        else:
            out[b, c - 2, 24:48] = y[::-1]
    return out


def kernel(**inputs):
    inp = {k: np.asarray(v) for k, v in inputs.items()}
    nc = _get_program()
    maps = make_in_maps(inp)
    res = run_bass_kernel_spmd(nc, maps, core_ids=list(range(8)), trace=False)
    return assemble(res.results)
